# revision 10
# baseline (speedup 1.0000x reference)
"""TRN2 Bass kernel for nn_BidirectionalAttention (B=4, T=1024, C=2048, 16 heads).

Sharding (8 cores): core c = 2*b + hg handles batch b, head-group hg (8 of 16
heads). Projections are tensor-parallel over heads; attention is fully local
per (batch, head); the output projection produces a partial (1024, 2048) sum
which is pairwise ReduceScatter-ed (cores 2b, 2b+1), after which each core
runs the 5-step LIF on its half (512, 2048) and returns it.

Layouts (host-prepped, contraction dim on partitions):
  xT  (C=2048, T=1024)    = x[b].T
  wqT/wkT/wvT (C, F=1024) = W.T[:, hg*1024:(hg+1)*1024]
  woT (F=1024, C=2048)    = Wo.T[hg*1024:(hg+1)*1024, :]
  cs  (128, T) = [cos.T; cos.T],  sn (128, T) = [sin.T; -sin.T]

Per head h on device: projection emits qT/kT in [d=128, T] (transposed)
layout; RoPE+rms_norm on DVE/ACT with the cross-partition sum-of-squares
done by an all-ones matmul (result broadcast to every partition, so no
partition-broadcast is ever needed); S.T = k @ q.T per (tk-chunk, tq-half);
exp on ACT (1/sqrt(128) folded into q's rms scale); softmax denominator =
all-ones matmul over the summed exp tiles; y.T = v.T @ P.T accumulated in
PSUM, normalized by the DVE-reciprocal of the denominator. v and y_att are
staged through DRAM to stay inside the 192KB/partition SBUF budget.
"""

import numpy as np

import concourse.bass as bass
import concourse.mybir as mybir
import concourse.tile as tile
from concourse import bacc
from concourse.alu_op_type import AluOpType
from concourse.bass_utils import run_bass_kernel_spmd

P = 128
B = 4
T = 1024
C = 2048
F = 1024          # local features = 8 heads x 128
NH = 8            # local heads
HD = 128
CO = C // P       # 16 contraction chunks for qkv projections
TQH = 2           # tq halves of 512
BETA = 0.9
THR = 1.0
STEPS = 5
EPS = 1e-6
N_CORES = 8

F32 = mybir.dt.float32
F32R = mybir.dt.float32r

# Precision knobs: dtype used for matmul operands per stage.
PROJ_DT = F32     # q/k/v projections
ATT_DT = F32      # qT/kT/v/expst for attention matmuls
OUT_DT = F32      # yT/woT for output projection
RMS_DT = F32      # squared tiles + ones for rms / denominator sums
DEN_VIA_PE = False  # True: softmax denom fully on PE (use with fp32r)

AFT = mybir.ActivationFunctionType

_CACHE = {}


def build(with_collective=True):
    nc = bacc.Bacc("TRN2", target_bir_lowering=False, debug=False,
                   num_devices=N_CORES)

    xT_d = nc.dram_tensor("xT", [C, T], PROJ_DT, kind="ExternalInput").ap()
    wqT_d = nc.dram_tensor("wqT", [C, F], PROJ_DT, kind="ExternalInput").ap()
    wkT_d = nc.dram_tensor("wkT", [C, F], PROJ_DT, kind="ExternalInput").ap()
    wvT_d = nc.dram_tensor("wvT", [C, F], PROJ_DT, kind="ExternalInput").ap()
    woT_d = nc.dram_tensor("woT", [F, C], OUT_DT, kind="ExternalInput").ap()
    cs_d = nc.dram_tensor("cs", [P, T], F32, kind="ExternalInput").ap()
    sn_d = nc.dram_tensor("sn", [P, T], F32, kind="ExternalInput").ap()
    ones_r_d = nc.dram_tensor("ones_r", [P, P], RMS_DT, kind="ExternalInput").ap()
    ones_a_d = nc.dram_tensor("ones_a", [P, P], ATT_DT, kind="ExternalInput").ap()
    bias_d = nc.dram_tensor("biases", [P, 2], F32, kind="ExternalInput").ap()
    out_d = nc.dram_tensor("out_half", [T // 2, C], F32,
                           kind="ExternalOutput").ap()

    xT_r = xT_d.rearrange("(co p) t -> p co t", p=P)
    wqT_r = wqT_d.rearrange("(co p) f -> p co f", p=P)
    wkT_r = wkT_d.rearrange("(co p) f -> p co f", p=P)
    wvT_r = wvT_d.rearrange("(co p) f -> p co f", p=P)
    woT_r = woT_d.rearrange("(fo p) c -> p fo c", p=P)

    with tile.TileContext(nc) as tc:
        with (
            tc.tile_pool(name="const", bufs=1) as const,
            tc.tile_pool(name="psum", bufs=1, space="PSUM") as psum,
            tc.tile_pool(name="dram", bufs=1, space="DRAM") as dram,
        ):
            cs_sb = const.tile([P, T], F32)
            sn_sb = const.tile([P, T], F32)
            ones_r = const.tile([P, P], RMS_DT)
            ones_a = const.tile([P, P], ATT_DT)
            bias_sb = const.tile([P, 2], F32)
            nc.sync.dma_start(cs_sb[:], cs_d)
            nc.sync.dma_start(sn_sb[:], sn_d)
            nc.sync.dma_start(ones_r[:], ones_r_d)
            nc.sync.dma_start(ones_a[:], ones_a_d)
            nc.sync.dma_start(bias_sb[:], bias_d)

            v_dram = dram.tile([T, F], ATT_DT)       # v, natural [t, f]
            v_r = v_dram.rearrange("(tc p) f -> p tc f", p=P)
            yT_dram = dram.tile([F, T], OUT_DT)      # y_att, [f, t] transposed
            yT_r = yT_dram.rearrange("(h p) t -> p h t", p=P)
            prered = dram.tile([T, C], F32)
            prered_r = prered.rearrange("(tc p) c -> p tc c", p=P)

            # ================= Phases 1 + 2 =================
            with (
                tc.tile_pool(name="xv", bufs=1) as xv,
                tc.tile_pool(name="work", bufs=2) as work,
                tc.tile_pool(name="att", bufs=1) as att,
                tc.tile_pool(name="exps", bufs=2) as exps,
                tc.tile_pool(name="vh", bufs=2) as vhp,
            ):
                xT_sb = xv.tile([P, CO, T], PROJ_DT)
                nc.sync.dma_start(xT_sb[:], xT_r)

                # ---- Phase 1a: v projection, natural [t, f] layout ----
                with tc.tile_pool(name="wv", bufs=1) as wvp:
                    for fh in range(2):
                        wv_sb = wvp.tile([P, CO, 512], PROJ_DT, tag="wv")
                        nc.sync.dma_start(
                            wv_sb[:], wvT_r[:, :, fh * 512:(fh + 1) * 512])
                        for tc_i in range(8):
                            ps = psum.tile([P, 512], F32, tag="hold", bufs=2)
                            for co in range(CO):
                                nc.tensor.matmul(
                                    ps[:],
                                    xT_sb[:, co, tc_i * 128:(tc_i + 1) * 128],
                                    wv_sb[:, co, :],
                                    start=(co == 0), stop=(co == CO - 1),
                                )
                            o = work.tile([P, 512], ATT_DT, tag="evict")
                            nc.vector.tensor_copy(o[:], ps[:])
                            nc.sync.dma_start(
                                v_r[:, tc_i, fh * 512:(fh + 1) * 512], o[:])

                # ---- Phases 1b + 2, per head ----
                with tc.tile_pool(name="wqk", bufs=3) as wqk:

                    def project_head(w_r, h, name):
                        w_sb = wqk.tile([P, CO, 128], PROJ_DT, tag="w")
                        nc.sync.dma_start(
                            w_sb[:], w_r[:, :, h * 128:(h + 1) * 128])
                        raw = work.tile([P, T], F32, tag="raw")
                        for th in range(TQH):
                            ps = psum.tile([P, 512], F32, tag="hold", bufs=2)
                            for co in range(CO):
                                nc.tensor.matmul(
                                    ps[:],
                                    w_sb[:, co, :],
                                    xT_sb[:, co, th * 512:(th + 1) * 512],
                                    start=(co == 0), stop=(co == CO - 1),
                                )
                            nc.vector.tensor_copy(
                                raw[:, th * 512:(th + 1) * 512], ps[:])
                        return raw

                    def rope_rms(raw, is_q, out_tag):
                        # rope: raw = raw*cs + swap(raw)*sn, in place
                        tmp = work.tile([P, T], F32, tag="tmp")
                        nc.vector.tensor_copy(tmp[0:64, :], raw[64:128, :])
                        nc.vector.tensor_copy(tmp[64:128, :], raw[0:64, :])
                        nc.vector.tensor_mul(raw[:], raw[:], cs_sb[:])
                        nc.vector.tensor_mul(tmp[:], tmp[:], sn_sb[:])
                        nc.vector.tensor_add(raw[:], raw[:], tmp[:])
                        # rms_norm over d (partitions) via all-ones matmul;
                        # Rsqrt ACT is banned -> Sqrt ACT + DVE reciprocal.
                        # q also folds the attention scale 1/sqrt(HD):
                        #   q*rsqrt(ss/HD+eps)/sqrt(HD) = q*rsqrt(ss + HD*eps)
                        sq = work.tile([P, T], RMS_DT, tag="tmp2")
                        nc.scalar.activation(sq[:], raw[:], AFT.Square)
                        sqv = work.tile([P, T], F32, tag="sqv")
                        for th in range(TQH):
                            ssp = psum.tile([P, 512], F32, tag="den", bufs=2)
                            nc.tensor.matmul(ssp[:], ones_r[:],
                                             sq[:, th * 512:(th + 1) * 512],
                                             start=True, stop=True)
                            if is_q:
                                nc.scalar.activation(
                                    sqv[:, th * 512:(th + 1) * 512], ssp[:],
                                    AFT.Sqrt, bias=bias_sb[:, 0:1], scale=1.0)
                            else:
                                nc.scalar.activation(
                                    sqv[:, th * 512:(th + 1) * 512], ssp[:],
                                    AFT.Sqrt, bias=bias_sb[:, 1:2],
                                    scale=float(1.0 / HD))
                        nc.vector.reciprocal(sqv[:], sqv[:])
                        out = att.tile([P, T], ATT_DT, tag=out_tag)
                        nc.vector.tensor_mul(out[:], raw[:], sqv[:])
                        return out

                    for h in range(NH):
                        qT = rope_rms(project_head(wqT_r, h, "q"), True, "qT")
                        kT = rope_rms(project_head(wkT_r, h, "k"), False, "kT")

                        vh = vhp.tile([P, NH, 128], ATT_DT, tag="vh")
                        nc.sync.dma_start(
                            vh[:], v_r[:, :, h * 128:(h + 1) * 128])

                        for th in range(TQH):
                            tq = slice(th * 512, (th + 1) * 512)
                            e_all = exps.tile([P, 8, 512], ATT_DT, tag="est")
                            for tkc in range(8):
                                stp = psum.tile([P, 512], F32, tag="st",
                                                bufs=3)
                                nc.tensor.matmul(
                                    stp[:],
                                    kT[:, tkc * 128:(tkc + 1) * 128],
                                    qT[:, tq],
                                    start=True, stop=True,
                                )
                                nc.scalar.activation(e_all[:, tkc, :], stp[:],
                                                     AFT.Exp)

                            # softmax denominator, broadcast to all partitions
                            denp = psum.tile([P, 512], F32, tag="den", bufs=2)
                            if DEN_VIA_PE:
                                for tkc in range(8):
                                    nc.tensor.matmul(
                                        denp[:], ones_a[:], e_all[:, tkc, :],
                                        start=(tkc == 0), stop=(tkc == 7))
                            else:
                                acc = work.tile([P, 512], F32, tag="denacc")
                                nc.vector.tensor_add(
                                    acc[:], e_all[:, 0, :].bitcast(F32),
                                    e_all[:, 1, :].bitcast(F32))
                                for tkc in range(2, 8):
                                    nc.vector.tensor_add(
                                        acc[:], acc[:],
                                        e_all[:, tkc, :].bitcast(F32))
                                nc.tensor.matmul(denp[:], ones_r[:],
                                                 acc[:].bitcast(RMS_DT),
                                                 start=True, stop=True)
                            rden = work.tile([P, 512], F32, tag="denacc2")
                            nc.vector.reciprocal(rden[:], denp[:])

                            yp = psum.tile([P, 512], F32, tag="hold", bufs=2)
                            for tkc in range(8):
                                nc.tensor.matmul(
                                    yp[:],
                                    vh[:, tkc, :],
                                    e_all[:, tkc, :],
                                    start=(tkc == 0), stop=(tkc == 7),
                                )
                            ynm = work.tile([P, 512], OUT_DT, tag="evict")
                            nc.vector.tensor_mul(ynm[:], yp[:], rden[:])
                            nc.sync.dma_start(yT_r[:, h, tq], ynm[:])

            # ======= Phase 3: output projection (partial over heads) =======
            with (
                tc.tile_pool(name="wo", bufs=2) as wop,
                tc.tile_pool(name="p3", bufs=2) as p3,
            ):
                for ch in range(4):
                    wo_sb = wop.tile([P, NH, 512], OUT_DT, tag="wo")
                    nc.sync.dma_start(
                        wo_sb[:], woT_r[:, :, ch * 512:(ch + 1) * 512])
                    for tc_i in range(8):
                        yt_sb = p3.tile([P, NH, 128], OUT_DT, tag="yt")
                        nc.sync.dma_start(
                            yt_sb[:],
                            yT_r[:, :, tc_i * 128:(tc_i + 1) * 128])
                        ps = psum.tile([P, 512], F32, tag="hold", bufs=2)
                        for h in range(NH):
                            nc.tensor.matmul(
                                ps[:],
                                yt_sb[:, h, :],
                                wo_sb[:, h, :],
                                start=(h == 0), stop=(h == NH - 1),
                            )
                        o = p3.tile([P, 512], F32, tag="osb")
                        nc.vector.tensor_copy(o[:], ps[:])
                        nc.sync.dma_start(
                            prered_r[:, tc_i, ch * 512:(ch + 1) * 512], o[:])

            # ======= Phase 4: pairwise ReduceScatter, then LIF =======
            rsout = dram.tile([T // 2, C], F32)
            if with_collective:
                nc.gpsimd.collective_compute(
                    "ReduceScatter",
                    AluOpType.add,
                    replica_groups=[[0, 1], [2, 3], [4, 5], [6, 7]],
                    ins=[prered[:]],
                    outs=[rsout[:]],
                )
            else:
                # timing-only stand-in (TimelineSim can't model collectives)
                nc.sync.dma_start(rsout[:], prered[: T // 2, :])

            rs_r = rsout.rearrange("(tc p) c -> p tc c", p=P)
            out_r = out_d.rearrange("(tc p) c -> p tc c", p=P)
            with tc.tile_pool(name="lif", bufs=1) as lif:
                for tc_i in range(4):
                    ysb = lif.tile([P, C], F32, tag="lify", bufs=2)
                    nc.sync.dma_start(ysb[:], rs_r[:, tc_i, :])
                    mem = lif.tile([P, C], F32, tag="lifmem")
                    acc = lif.tile([P, C], F32, tag="lifacc")
                    spk = lif.tile([P, C], F32, tag="lifspk")
                    # step 1: mem = y; spk = (y > thr); mem -= thr*spk
                    nc.vector.tensor_scalar(out=acc[:], in0=ysb[:],
                                            scalar1=THR, scalar2=None,
                                            op0=AluOpType.is_gt)
                    nc.vector.scalar_tensor_tensor(
                        out=mem[:], in0=acc[:], scalar=-THR, in1=ysb[:],
                        op0=AluOpType.mult, op1=AluOpType.add)
                    for _ in range(STEPS - 2):
                        nc.vector.scalar_tensor_tensor(
                            out=mem[:], in0=mem[:], scalar=BETA, in1=ysb[:],
                            op0=AluOpType.mult, op1=AluOpType.add)
                        nc.vector.tensor_scalar(out=spk[:], in0=mem[:],
                                                scalar1=THR, scalar2=None,
                                                op0=AluOpType.is_gt)
                        nc.vector.tensor_add(acc[:], acc[:], spk[:])
                        nc.vector.scalar_tensor_tensor(
                            out=mem[:], in0=spk[:], scalar=-THR, in1=mem[:],
                            op0=AluOpType.mult, op1=AluOpType.add)
                    # last step: only the spike matters; fuse compare+accum
                    nc.vector.scalar_tensor_tensor(
                        out=mem[:], in0=mem[:], scalar=BETA, in1=ysb[:],
                        op0=AluOpType.mult, op1=AluOpType.add)
                    nc.vector.scalar_tensor_tensor(
                        out=acc[:], in0=mem[:], scalar=THR, in1=acc[:],
                        op0=AluOpType.is_gt, op1=AluOpType.add)
                    nc.vector.tensor_scalar_mul(acc[:], acc[:], 1.0 / STEPS)
                    nc.sync.dma_start(out_r[:, tc_i, :], acc[:])

    nc.compile()
    return nc


def prep_in_maps(x, cos, sin, Wq, Wk, Wv, Wo):
    x = np.asarray(x, np.float32)
    cosT = np.ascontiguousarray(np.asarray(cos, np.float32)[0, :, 0, :].T)
    sinT = np.ascontiguousarray(np.asarray(sin, np.float32)[0, :, 0, :].T)
    cs = np.concatenate([cosT, cosT], axis=0)          # (128, T)
    sn = np.concatenate([sinT, -sinT], axis=0)         # (128, T)
    WqT = np.ascontiguousarray(np.asarray(Wq, np.float32).T)
    WkT = np.ascontiguousarray(np.asarray(Wk, np.float32).T)
    WvT = np.ascontiguousarray(np.asarray(Wv, np.float32).T)
    WoT = np.ascontiguousarray(np.asarray(Wo, np.float32).T)
    ones = np.ones((P, P), np.float32)
    biases = np.empty((P, 2), np.float32)
    biases[:, 0] = HD * EPS
    biases[:, 1] = EPS

    in_maps = []
    for c in range(N_CORES):
        b, hg = c // 2, c % 2
        fs = slice(hg * F, (hg + 1) * F)
        in_maps.append({
            "xT": np.ascontiguousarray(x[b].T),
            "wqT": np.ascontiguousarray(WqT[:, fs]),
            "wkT": np.ascontiguousarray(WkT[:, fs]),
            "wvT": np.ascontiguousarray(WvT[:, fs]),
            "woT": np.ascontiguousarray(WoT[fs, :]),
            "cs": cs, "sn": sn,
            "ones_r": ones, "ones_a": ones,
            "biases": biases,
        })
    return in_maps


def kernel(x, cos, sin, Wq, Wk, Wv, Wo):
    if "nc" not in _CACHE:
        _CACHE["nc"] = build()
    nc = _CACHE["nc"]

    in_maps = prep_in_maps(x, cos, sin, Wq, Wk, Wv, Wo)
    res = run_bass_kernel_spmd(nc, in_maps, core_ids=list(range(N_CORES)))
    _CACHE["last_res"] = res

    out = np.empty((B, T, C), np.float32)
    for c in range(N_CORES):
        b, hg = c // 2, c % 2
        out[b, hg * 512:(hg + 1) * 512, :] = res.results[c]["out_half"]
    return out


# revision 28
# speedup vs baseline: 32.3937x; 32.3937x over previous
"""TRN2 Bass kernel for nn_BidirectionalAttention (B=4, T=1024, C=2048, 16 heads).

Sharding (8 cores): core c = 2*b + hg handles batch b, head-group hg (8 of 16
heads). Projections are tensor-parallel over heads; attention is fully local
per (batch, head); the output projection produces a partial (1024, 2048) sum
which is pairwise ReduceScatter-ed (cores 2b, 2b+1), after which each core
runs the 5-step LIF on its half (512, 2048) and returns it.

Layouts (host-prepped, contraction dim on partitions):
  xT  (C=2048, T=1024)    = x[b].T
  wqT/wkT/wvT (C, F=1024) = W.T[:, hg*1024:(hg+1)*1024]
  woT (F=1024, C=2048)    = Wo.T[hg*1024:(hg+1)*1024, :]
  cs  (128, T) = [cos.T; cos.T],  sn (128, T) = [sin.T; -sin.T]

Per head h on device: projection emits qT/kT in [d=128, T] (transposed)
layout; RoPE+rms_norm on DVE/ACT with the cross-partition sum-of-squares
done by an all-ones matmul (result broadcast to every partition, so no
partition-broadcast is ever needed); S.T = k @ q.T per (tk-chunk, tq-half);
exp on ACT (1/sqrt(128) folded into q's rms scale); softmax denominator =
all-ones matmul over the summed exp tiles; y.T = v.T @ P.T accumulated in
PSUM, normalized by the DVE-reciprocal of the denominator. v and y_att are
staged through DRAM to stay inside the 192KB/partition SBUF budget.

build(reps=N) repeats the whole pipeline N times inside one NEFF (for
wall-clock benching); upto in {"v", "heads", "wo", "full"} truncates phases
(for cost-model ablation).
"""

import numpy as np

import concourse.bass as bass
import concourse.mybir as mybir
import concourse.tile as tile
from concourse import bacc
from concourse.alu_op_type import AluOpType
from concourse.bass_utils import run_bass_kernel_spmd

P = 128
B = 4
T = 1024
C = 2048
F = 1024          # local features = 8 heads x 128
NH = 8            # local heads
HD = 128
CO = C // P       # 16 contraction chunks for qkv projections
TQH = 2           # tq halves of 512
BETA = 0.9
THR = 1.0
STEPS = 5
EPS = 1e-6
N_CORES = 8

F32 = mybir.dt.float32
F32R = mybir.dt.float32r

# Precision knobs: dtype used for matmul operands per stage.
PROJ_DT = F32     # q/k/v projections
ATT_DT = F32      # qT/kT/v/expst for attention matmuls
OUT_DT = F32      # yT/woT for output projection
RMS_DT = F32      # squared tiles + ones for rms / denominator sums
DEN_VIA_PE = False  # True: softmax denom fully on PE (use with fp32r)

AFT = mybir.ActivationFunctionType

_CACHE = {}


def build(with_collective=True, reps=1, upto="full"):
    nc = bacc.Bacc("TRN2", target_bir_lowering=False, debug=False,
                   num_devices=N_CORES)

    xT_d = nc.dram_tensor("xT", [C, T], PROJ_DT, kind="ExternalInput").ap()
    wqT_d = nc.dram_tensor("wqT", [C, F], PROJ_DT, kind="ExternalInput").ap()
    wkT_d = nc.dram_tensor("wkT", [C, F], PROJ_DT, kind="ExternalInput").ap()
    wvT_d = nc.dram_tensor("wvT", [C, F], PROJ_DT, kind="ExternalInput").ap()
    woT_d = nc.dram_tensor("woT", [F, C], OUT_DT, kind="ExternalInput").ap()
    cs_d = nc.dram_tensor("cs", [P, T], F32, kind="ExternalInput").ap()
    sn_d = nc.dram_tensor("sn", [P, T], F32, kind="ExternalInput").ap()
    ones_r_d = nc.dram_tensor("ones_r", [P, P], RMS_DT, kind="ExternalInput").ap()
    ones_a_d = nc.dram_tensor("ones_a", [P, P], ATT_DT, kind="ExternalInput").ap()
    ones_d_d = nc.dram_tensor("ones_d", [P, P], F32, kind="ExternalInput").ap()
    bias_d = nc.dram_tensor("biases", [P, 2], F32, kind="ExternalInput").ap()
    out_d = nc.dram_tensor("out_half", [2, T, 512], F32,
                           kind="ExternalOutput").ap()

    xT_r = xT_d.rearrange("(co p) t -> p co t", p=P)
    wqT_r = wqT_d.rearrange("(co p) f -> p co f", p=P)
    wkT_r = wkT_d.rearrange("(co p) f -> p co f", p=P)
    wvT_r = wvT_d.rearrange("(co p) f -> p co f", p=P)
    woT_r = woT_d.rearrange("(fo p) c -> p fo c", p=P)

    with tile.TileContext(nc) as tc:
        with (
            tc.tile_pool(name="const", bufs=1) as const,
            tc.tile_pool(name="psum", bufs=1, space="PSUM") as psum,
            tc.tile_pool(name="dram", bufs=1, space="DRAM") as dram,
        ):
            cs_sb = const.tile([P, T], F32)
            sn_sb = const.tile([P, T], F32)
            ones_r = const.tile([P, P], RMS_DT)
            ones_a = const.tile([P, P], ATT_DT)
            ones_d = const.tile([P, P], F32)
            bias_sb = const.tile([P, 2], F32)
            nc.sync.dma_start(cs_sb[:], cs_d)
            nc.sync.dma_start(sn_sb[:], sn_d)
            nc.sync.dma_start(ones_r[:], ones_r_d)
            nc.sync.dma_start(ones_a[:], ones_a_d)
            nc.sync.dma_start(ones_d[:], ones_d_d)
            nc.sync.dma_start(bias_sb[:], bias_d)

            v_dram = dram.tile([T, F], ATT_DT)       # v, natural [t, f]
            v_r = v_dram.rearrange("(tc p) f -> p tc f", p=P)
            yT_dram = dram.tile([F, T], OUT_DT)      # y_att, [f, t] transposed
            yT_r = yT_dram.rearrange("(h p) t -> p h t", p=P)
            # ch-major partial sums: [ch, t, 512] so each ch block is
            # contiguous and can ReduceScatter as soon as it completes
            prered = dram.tile([4, T, 512], F32)
            prered_r = prered.rearrange("ch (tc p) c -> p ch tc c", p=P)
            # single RS shards the flat [4,T,512] prered: rank0 gets ch
            # blocks 0,1 (= columns 0:1024 for all t), rank1 blocks 2,3
            rsout = dram.tile([2, T, 512], F32)
            rs_r = rsout.rearrange("ci (tc p) c -> p ci tc c", p=P)
            out_r = out_d.rearrange("ci (tc p) c -> p ci tc c", p=P)

            for rep in range(reps):
                _emit_rep(nc, tc, rep, upto, with_collective, psum,
                          xT_r, wqT_r, wkT_r, wvT_r, woT_r,
                          cs_sb, sn_sb, ones_r, ones_a, ones_d, bias_sb,
                          v_r, yT_r, prered, prered_r, rsout, rs_r, out_r)

    nc.compile()
    return nc


def _emit_rep(nc, tc, rep, upto, with_collective, psum,
              xT_r, wqT_r, wkT_r, wvT_r, woT_r,
              cs_sb, sn_sb, ones_r, ones_a, ones_d, bias_sb,
              v_r, yT_r, prered, prered_r, rsout, rs_r, out_r):
    # ================= Phases 1 + 2 =================
    with (
        tc.tile_pool(name=f"xv{rep}", bufs=1) as xv,
        tc.tile_pool(name=f"work{rep}", bufs=2) as work,
        tc.tile_pool(name=f"att{rep}", bufs=2) as att,
        tc.tile_pool(name=f"exps{rep}", bufs=2) as exps,
        tc.tile_pool(name=f"vh{rep}", bufs=2) as vhp,
    ):
        xT_sb = xv.tile([P, CO, T], PROJ_DT)
        for co in range(CO):   # split across DMA queues
            nc.sync.dma_start(xT_sb[:, co, :], xT_r[:, co, :])

        # ---- Phase 1a: v projection, natural [t, f] layout ----
        with tc.tile_pool(name=f"wv{rep}", bufs=1) as wvp:
            for fh in range(2):
                wv_sb = wvp.tile([P, CO, 512], PROJ_DT, tag="wv")
                for co in range(CO):
                    nc.sync.dma_start(
                        wv_sb[:, co, :],
                        wvT_r[:, co, fh * 512:(fh + 1) * 512])
                for tc_i in range(8):
                    ps = psum.tile([P, 512], F32, tag="hold", bufs=3)
                    for co in range(CO):
                        nc.tensor.matmul(
                            ps[:],
                            xT_sb[:, co, tc_i * 128:(tc_i + 1) * 128],
                            wv_sb[:, co, :],
                            start=(co == 0), stop=(co == CO - 1),
                        )
                    o = work.tile([P, 512], ATT_DT, tag="evict")
                    nc.vector.tensor_copy(o[:], ps[:])
                    nc.sync.dma_start(
                        v_r[:, tc_i, fh * 512:(fh + 1) * 512], o[:])

        if upto == "v":
            return

        # ---- Phases 1b + 2, per head ----
        with tc.tile_pool(name=f"wqk{rep}", bufs=3) as wqk:

            def project_head(w_r, h):
                w_sb = wqk.tile([P, CO, 128], PROJ_DT, tag="w")
                for cg in range(4):   # split across DMA queues
                    nc.sync.dma_start(
                        w_sb[:, cg * 4:(cg + 1) * 4, :],
                        w_r[:, cg * 4:(cg + 1) * 4, h * 128:(h + 1) * 128])
                raw = work.tile([P, T], F32, tag="raw")
                for th in range(TQH):
                    ps = psum.tile([P, 512], F32, tag="hold", bufs=3)
                    for co in range(CO):
                        nc.tensor.matmul(
                            ps[:],
                            w_sb[:, co, :],
                            xT_r_sb[:, co, th * 512:(th + 1) * 512],
                            start=(co == 0), stop=(co == CO - 1),
                        )
                    nc.vector.tensor_copy(
                        raw[:, th * 512:(th + 1) * 512], ps[:])
                return raw

            xT_r_sb = xT_sb  # closure alias

            def rope_rms(raw, is_q, out_tag):
                # rope: raw = raw*cs + swap(raw)*sn, in place
                tmp = work.tile([P, T], F32, tag="tmp")
                nc.vector.tensor_copy(tmp[0:64, :], raw[64:128, :])
                nc.vector.tensor_copy(tmp[64:128, :], raw[0:64, :])
                nc.vector.tensor_mul(raw[:], raw[:], cs_sb[:])
                nc.vector.tensor_mul(tmp[:], tmp[:], sn_sb[:])
                nc.vector.tensor_add(raw[:], raw[:], tmp[:])
                # rms_norm over d (partitions) via all-ones matmul;
                # Rsqrt ACT is banned -> Sqrt ACT + DVE reciprocal.
                # q also folds the attention scale 1/sqrt(HD):
                #   q*rsqrt(ss/HD+eps)/sqrt(HD) = q*rsqrt(ss + HD*eps)
                sq = work.tile([P, T], RMS_DT, tag="tmp2")
                nc.scalar.activation(sq[:], raw[:], AFT.Square)
                sqv = work.tile([P, T], F32, tag="sqv")
                for th in range(TQH):
                    ssp = psum.tile([P, 512], F32, tag="den", bufs=2)
                    nc.tensor.matmul(ssp[:], ones_r[:],
                                     sq[:, th * 512:(th + 1) * 512],
                                     start=True, stop=True)
                    if is_q:
                        nc.scalar.activation(
                            sqv[:, th * 512:(th + 1) * 512], ssp[:],
                            AFT.Sqrt, bias=bias_sb[:, 0:1], scale=1.0)
                    else:
                        nc.scalar.activation(
                            sqv[:, th * 512:(th + 1) * 512], ssp[:],
                            AFT.Sqrt, bias=bias_sb[:, 1:2],
                            scale=float(1.0 / HD))
                nc.vector.reciprocal(sqv[:], sqv[:])
                out = att.tile([P, T], ATT_DT, tag=out_tag)
                nc.vector.tensor_mul(out[:], raw[:], sqv[:])
                return out

            # software-pipelined: head h+1's projections are emitted before
            # head h's attention so the PE never waits on a rope/rms chain
            qT = rope_rms(project_head(wqT_r, 0), True, "qT")
            kT = rope_rms(project_head(wkT_r, 0), False, "kT")
            for h in range(NH):
                if h + 1 < NH:
                    qT_next = rope_rms(project_head(wqT_r, h + 1), True, "qT")
                    kT_next = rope_rms(project_head(wkT_r, h + 1), False, "kT")

                vh = vhp.tile([P, NH, 128], ATT_DT, tag="vh")
                nc.sync.dma_start(vh[:], v_r[:, :, h * 128:(h + 1) * 128])

                for th in range(TQH):
                    tq = slice(th * 512, (th + 1) * 512)
                    e_all = exps.tile([P, 8, 512], ATT_DT, tag="est")
                    for tkc in range(8):
                        stp = psum.tile([P, 512], F32, tag="st", bufs=3)
                        nc.tensor.matmul(
                            stp[:],
                            kT[:, tkc * 128:(tkc + 1) * 128],
                            qT[:, tq],
                            start=True, stop=True,
                        )
                        nc.scalar.activation(e_all[:, tkc, :], stp[:],
                                             AFT.Exp)

                    # softmax denominator, broadcast to all partitions
                    denp = psum.tile([P, 512], F32, tag="den", bufs=2)
                    if DEN_VIA_PE:
                        for tkc in range(8):
                            nc.tensor.matmul(
                                denp[:], ones_a[:], e_all[:, tkc, :],
                                start=(tkc == 0), stop=(tkc == 7))
                    else:
                        acc = work.tile([P, 512], F32, tag="denacc")
                        nc.vector.tensor_add(
                            acc[:], e_all[:, 0, :].bitcast(F32),
                            e_all[:, 1, :].bitcast(F32))
                        for tkc in range(2, 8):
                            nc.vector.tensor_add(
                                acc[:], acc[:],
                                e_all[:, tkc, :].bitcast(F32))
                        nc.tensor.matmul(denp[:], ones_d[:], acc[:],
                                         start=True, stop=True)
                    rden = work.tile([P, 512], F32, tag="denacc2")
                    nc.vector.reciprocal(rden[:], denp[:])

                    yp = psum.tile([P, 512], F32, tag="hold", bufs=3)
                    for tkc in range(8):
                        nc.tensor.matmul(
                            yp[:],
                            vh[:, tkc, :],
                            e_all[:, tkc, :],
                            start=(tkc == 0), stop=(tkc == 7),
                        )
                    ynm = work.tile([P, 512], OUT_DT, tag="evict")
                    nc.vector.tensor_mul(ynm[:], yp[:], rden[:])
                    nc.sync.dma_start(yT_r[:, h, tq], ynm[:])

                if h + 1 < NH:
                    qT, kT = qT_next, kT_next

    if upto == "heads":
        return

    # ======= Phase 3 + 4: output projection, interleaved with chunked
    # ReduceScatter + LIF. ch-outer so each contiguous prered[ch] block
    # completes early; its RS and LIF overlap the next ch's matmuls. =======
    with (
        tc.tile_pool(name=f"wo{rep}", bufs=2) as wop,
        tc.tile_pool(name=f"p3{rep}", bufs=2) as p3,
        tc.tile_pool(name=f"lif{rep}", bufs=1) as lif,
    ):
        for ch in range(4):
            wo_sb = wop.tile([P, NH, 512], OUT_DT, tag="wo")
            for h in range(NH):
                nc.sync.dma_start(
                    wo_sb[:, h, :], woT_r[:, h, ch * 512:(ch + 1) * 512])
            for tc_i in range(8):
                yt_sb = p3.tile([P, NH, 128], OUT_DT, tag="yt")
                nc.sync.dma_start(
                    yt_sb[:], yT_r[:, :, tc_i * 128:(tc_i + 1) * 128])
                ps = psum.tile([P, 512], F32, tag="hold", bufs=3)
                for h in range(NH):
                    nc.tensor.matmul(
                        ps[:],
                        yt_sb[:, h, :],
                        wo_sb[:, h, :],
                        start=(h == 0), stop=(h == NH - 1),
                    )
                o = p3.tile([P, 512], F32, tag="osb")
                nc.vector.tensor_copy(o[:], ps[:])
                nc.sync.dma_start(prered_r[:, ch, tc_i, :], o[:])

        if upto == "wo":
            return

        # ======= one pairwise ReduceScatter over the whole partial =======
        if with_collective:
            nc.gpsimd.collective_compute(
                "ReduceScatter",
                AluOpType.add,
                replica_groups=[[0, 1], [2, 3], [4, 5], [6, 7]],
                ins=[prered[:]],
                outs=[rsout[:]],
            )
        else:
            # timing-only stand-in (TimelineSim lacks collectives)
            nc.sync.dma_start(rsout[:], prered[:2])

        for ci in range(2):
          for th in range(2):
            # LIF chunk: [p, tc=4, 512] = 2048 elems/lane
            ysb = lif.tile([P, 4, 512], F32, tag="lify", bufs=2)
            nc.sync.dma_start(ysb[:], rs_r[:, ci, th * 4:(th + 1) * 4, :])
            mem = lif.tile([P, 4, 512], F32, tag="lifmem", bufs=2)
            acc = lif.tile([P, 4, 512], F32, tag="lifacc", bufs=2)
            spk = lif.tile([P, 4, 512], F32, tag="lifspk", bufs=2)
            # step 1: mem = y; spk = (y > thr); mem -= thr*spk
            nc.vector.tensor_scalar(out=acc[:], in0=ysb[:],
                                    scalar1=THR, scalar2=None,
                                    op0=AluOpType.is_gt)
            nc.vector.scalar_tensor_tensor(
                out=mem[:], in0=acc[:], scalar=-THR, in1=ysb[:],
                op0=AluOpType.mult, op1=AluOpType.add)
            for _ in range(STEPS - 2):
                nc.vector.scalar_tensor_tensor(
                    out=mem[:], in0=mem[:], scalar=BETA, in1=ysb[:],
                    op0=AluOpType.mult, op1=AluOpType.add)
                nc.vector.tensor_scalar(out=spk[:], in0=mem[:],
                                        scalar1=THR, scalar2=None,
                                        op0=AluOpType.is_gt)
                nc.vector.tensor_add(acc[:], acc[:], spk[:])
                nc.vector.scalar_tensor_tensor(
                    out=mem[:], in0=spk[:], scalar=-THR, in1=mem[:],
                    op0=AluOpType.mult, op1=AluOpType.add)
            # last step: only the spike matters; fuse compare+accumulate
            nc.vector.scalar_tensor_tensor(
                out=mem[:], in0=mem[:], scalar=BETA, in1=ysb[:],
                op0=AluOpType.mult, op1=AluOpType.add)
            nc.vector.scalar_tensor_tensor(
                out=acc[:], in0=mem[:], scalar=THR, in1=acc[:],
                op0=AluOpType.is_gt, op1=AluOpType.add)
            nc.vector.tensor_scalar_mul(acc[:], acc[:], 1.0 / STEPS)
            nc.sync.dma_start(out_r[:, ci, th * 4:(th + 1) * 4, :], acc[:])


def prep_in_maps(x, cos, sin, Wq, Wk, Wv, Wo):
    x = np.asarray(x, np.float32)
    cosT = np.ascontiguousarray(np.asarray(cos, np.float32)[0, :, 0, :].T)
    sinT = np.ascontiguousarray(np.asarray(sin, np.float32)[0, :, 0, :].T)
    cs = np.concatenate([cosT, cosT], axis=0)          # (128, T)
    sn = np.concatenate([sinT, -sinT], axis=0)         # (128, T)
    WqT = np.ascontiguousarray(np.asarray(Wq, np.float32).T)
    WkT = np.ascontiguousarray(np.asarray(Wk, np.float32).T)
    WvT = np.ascontiguousarray(np.asarray(Wv, np.float32).T)
    WoT = np.ascontiguousarray(np.asarray(Wo, np.float32).T)
    ones = np.ones((P, P), np.float32)
    biases = np.empty((P, 2), np.float32)
    biases[:, 0] = HD * EPS
    biases[:, 1] = EPS

    in_maps = []
    for c in range(N_CORES):
        b, hg = c // 2, c % 2
        fs = slice(hg * F, (hg + 1) * F)
        in_maps.append({
            "xT": np.ascontiguousarray(x[b].T),
            "wqT": np.ascontiguousarray(WqT[:, fs]),
            "wkT": np.ascontiguousarray(WkT[:, fs]),
            "wvT": np.ascontiguousarray(WvT[:, fs]),
            "woT": np.ascontiguousarray(WoT[fs, :]),
            "cs": cs, "sn": sn,
            "ones_r": ones, "ones_a": ones, "ones_d": ones,
            "biases": biases,
        })
    return in_maps


def kernel(x, cos, sin, Wq, Wk, Wv, Wo):
    if "nc" not in _CACHE:
        _CACHE["nc"] = build()
    nc = _CACHE["nc"]

    in_maps = prep_in_maps(x, cos, sin, Wq, Wk, Wv, Wo)
    res = run_bass_kernel_spmd(nc, in_maps, core_ids=list(range(N_CORES)))
    _CACHE["last_res"] = res

    # out_half is [2, T, 512]: rank hg of pair b holds column blocks
    # (2*hg + ci) for all T rows of batch b.
    out = np.empty((B, T, C), np.float32)
    for c in range(N_CORES):
        b, hg = c // 2, c % 2
        oh = res.results[c]["out_half"]
        for ci in range(2):
            cc = (2 * hg + ci) * 512
            out[b, :, cc:cc + 512] = oh[ci]
    return out
